# revision 54
# baseline (speedup 1.0000x reference)
"""CondLaneHead DynamicMaskHead kernel for 8 Trainium2 NeuronCores.

Problem: per-instance 3-layer 1x1-conv MLP over a [64,160,256] feature map.
  feats = concat([loc_x, loc_y], x[img])            # [66, L], L = 160*256
  h1 = relu(w0 @ feats + b0)                        # [64, L]
  h2 = relu(w1 @ h1 + b1)                           # [64, L]
  out = w2 @ h2 + b2 - 2.19                         # [1, L]
32 instances (8 per image, 4 images).

Sharding (hybrid): core c -> image c//2, position half c%2. Each core runs
all 8 instances of its image over Lc = L/2 = 20480 positions, so every byte
of x is shipped to exactly one core.

Device mapping (per core, 8 instances = 4 pairs, 20 chunks of T=1024):
  - feats live in 10 resident SBUF tiles [66, 2048] bf16: rows 0-63 = x
    chunk (64-row DMA -> spreads across all 16 SDMA engines; odd partition
    counts serialize onto one engine at ~26 GB/s), row 64 = loc_x pattern,
    row 65 = (col//256) pattern (both periodic with 1024, loaded once).
    The chunk-dependent part of the loc_y term, w0y*(80*half + 4*i), is
    folded into a per-chunk L1 relu bias together with b0.
  - L1: per pair one [66,128] bf16 lhsT (2 instances side by side), two
    512-col matmuls per chunk into a [128,1024] PSUM tile.
  - L2: block-diagonal [128,128] bf16 weights per pair.
  - L3 (64->1): outputs packed across PSUM partitions: per 512-position
    group one matmul with a zero-padded [128,32] w2 slice writes a [32,512]
    window (tile_position), accumulating 16 groups per window, so one
    [128,512] PSUM bank collects 64 groups before a single bias+copy op
    and a few strided DMAs to DRAM.
  - relu work (PSUM->SBUF copies) is split greedily between ACT and DVE.
"""

import sys

if "/opt/trn_rl_repo" not in sys.path:
    sys.path.insert(0, "/opt/trn_rl_repo")

import numpy as np
import ml_dtypes

import concourse.bass as bass
import concourse.mybir as mybir
from concourse import bacc
from concourse.tile import TileContext
from concourse.bass_utils import run_bass_kernel_spmd

BF = mybir.dt.bfloat16
F32 = mybir.dt.float32
AT = mybir.ActivationFunctionType
AL = mybir.AluOpType
bf16 = ml_dtypes.bfloat16

# Problem geometry (hardcoded per spec)
N_IMG, INS_PER_IMG, C, H, W = 4, 8, 64, 160, 256
CIN = C + 2
L = H * W                       # 40960 positions per image
L1, L2_, L3_ = (C + 2) * C, C * C, C
B1OFF = L1 + L2_ + L3_          # offsets into the 8513-param vector
MASK_BIAS_SHIFT = 2.19

N_CORES = 8
NPAIR = 4                       # 8 instances per core, 2 per matmul pack
LC = L // 2                     # 20480 positions per core
T = 1024                        # positions per chunk
NCHUNK = LC // T                # 20
FT = 5120                       # positions per feats SBUF tile
NFT = LC // FT                  # 4 resident feats tiles
CPF = FT // T                   # 5 chunks per feats tile
GROUPS = NCHUNK * NPAIR * 2     # 160 [2, 512] position-groups per core
N_BANKS = (GROUPS + 63) // 64   # 3 output PSUM bank fills (64, 64, 32)

# relu op cost estimates (ns, HW-measured) for greedy ACT/DVE balancing
COST_DVE = 1258.0
COST_ACT = 1165.0

_cache = {}


def _build_program():
    nc = bacc.Bacc("TRN2", target_bir_lowering=False, debug=False)

    # batched inputs: DMA-issue costs ~0.6us each on the SP queue, so ship
    # few big tensors. xb rows 64/65 carry the coord patterns. wbf packs
    # w1 (4x128 cols) then w2pad (4x512 cols); wf32 packs b0 (4xNCHUNK),
    # b1 (4x1), b2 (1).
    xb = nc.dram_tensor("xb", [CIN, LC], BF, kind="ExternalInput")
    w0c = nc.dram_tensor("w0c", [CIN, NPAIR * 128], BF, kind="ExternalInput")
    wbf = nc.dram_tensor("wbf", [128, NPAIR * (128 + 512)], BF,
                         kind="ExternalInput")
    wf32 = nc.dram_tensor("wf32", [128, NPAIR * (NCHUNK + 1) + 1], F32,
                          kind="ExternalInput")
    # packed output: [bank, q, col]; host un-permutes (q encodes
    # chunk/pair/half/instance) — keeps each flush one big contiguous DMA.
    o = nc.dram_tensor("o", [N_BANKS * 128, 512], F32, kind="ExternalOutput")

    eng_ns = {"dve": 0.0, "act": 0.0}

    def relu(dst, src, bias_ap):
        if eng_ns["dve"] + COST_DVE <= eng_ns["act"] + COST_ACT:
            eng_ns["dve"] += COST_DVE
            if bias_ap is None:
                nc.vector.tensor_scalar(out=dst, in0=src, scalar1=0.0,
                                        scalar2=None, op0=AL.max)
            else:
                nc.vector.tensor_scalar(out=dst, in0=src, scalar1=bias_ap,
                                        scalar2=0.0, op0=AL.add, op1=AL.max)
        else:
            eng_ns["act"] += COST_ACT
            if bias_ap is None:
                nc.scalar.activation(dst, src, AT.Relu)
            else:
                nc.scalar.activation(dst, src, AT.Relu, bias=bias_ap)

    with TileContext(nc) as tc:
        with tc.tile_pool(name="consts", bufs=1) as cpool, \
             tc.tile_pool(name="zpool", bufs=3, space="PSUM") as zpool, \
             tc.tile_pool(name="opool", bufs=2, space="PSUM") as opool, \
             tc.tile_pool(name="h1pool", bufs=10) as h1pool, \
             tc.tile_pool(name="h2pool", bufs=16) as h2pool, \
             tc.tile_pool(name="spool", bufs=2) as spool:

            # PE warm-up: dummy matmuls on a zeroed tile fill the idle
            # window while the first DMAs land, so the p-state ramp
            # (0.65/1.2 GHz until ~3us of continuous execution) happens on
            # throwaway work instead of the first real chunks.
            warm = cpool.tile([128, 512], BF, name="warm")
            nc.vector.memset(warm, 0.0)
            wz = zpool.tile([128, T], F32, name="warmz", tag="z")
            for k in range(7):
                nc.tensor.matmul(wz[:, 0:512], warm[:, 0:128], warm,
                                 start=True, stop=True)

            # DMA order: what the first chunk needs first (w0c + a small
            # first feats piece), then relu bias (wf32), then L2/L3 weights,
            # then the rest of the feats. All on the sync ring — issuing
            # from the scalar ring delays ACT's relu work (measured slower).
            w0c_sb = cpool.tile([CIN, NPAIR * 128], BF, name="w0csb")
            nc.sync.dma_start(out=w0c_sb, in_=w0c.ap())
            ft0a = cpool.tile([CIN, T], BF, name="ft0a")
            nc.sync.dma_start(out=ft0a, in_=xb.ap()[:, 0:T])
            wf32_sb = cpool.tile([128, NPAIR * (NCHUNK + 1) + 1], F32,
                                 name="wf32sb")
            nc.sync.dma_start(out=wf32_sb, in_=wf32.ap())
            ft0b = cpool.tile([CIN, FT - T], BF, name="ft0b")
            nc.sync.dma_start(out=ft0b, in_=xb.ap()[:, T:FT])
            wbf_sb = cpool.tile([128, NPAIR * (128 + 512)], BF, name="wbfsb")
            nc.sync.dma_start(out=wbf_sb, in_=wbf.ap())
            w0_sb = [w0c_sb[:, p * 128:(p + 1) * 128] for p in range(NPAIR)]
            w1_sb = [wbf_sb[:, p * 128:(p + 1) * 128] for p in range(NPAIR)]
            w2_sb = [wbf_sb[:, NPAIR * 128 + p * 512:
                            NPAIR * 128 + (p + 1) * 512]
                     for p in range(NPAIR)]
            b0_sb = [wf32_sb[:, p * NCHUNK:(p + 1) * NCHUNK]
                     for p in range(NPAIR)]
            b1_sb = [wf32_sb[:, NPAIR * NCHUNK + p:NPAIR * NCHUNK + p + 1]
                     for p in range(NPAIR)]
            b2_sb = wf32_sb[:, NPAIR * (NCHUNK + 1):
                            NPAIR * (NCHUNK + 1) + 1]

            # remaining resident feats tiles (x + coord rows in one DMA each)
            fts = [None]
            for j in range(1, NFT):
                ft = cpool.tile([CIN, FT], BF, name=f"ft{j}")
                nc.sync.dma_start(out=ft,
                                  in_=xb.ap()[:, j * FT:(j + 1) * FT])
                fts.append(ft)

            def feat_slice(i, s):
                c = i * T + s * 512
                if c < T:
                    return ft0a[:, c:c + 512]
                if c < FT:
                    return ft0b[:, c - T:c - T + 512]
                return fts[i // CPF][:, c - (i // CPF) * FT:
                                     c - (i // CPF) * FT + 512]

            z1s, z2s, h1s, h2s = {}, {}, {}, {}
            obank = {"tile": None, "idx": -1}

            def flush_obank(row0, nrows):
                ob = obank["tile"]
                b = obank["idx"]
                stage = spool.tile([128, 512], F32, name=f"stage{b}_{row0}",
                                   tag="stage")
                nc.scalar.activation(stage[row0:row0 + nrows],
                                     ob[row0:row0 + nrows], AT.Identity,
                                     bias=b2_sb[row0:row0 + nrows])
                eng_ns["act"] += 750.0
                dst = bass.AP(o, (b * 128 + row0) * 512,
                              [[512, nrows], [1, 512]])
                nc.sync.dma_start(out=dst, in_=stage[row0:row0 + nrows])

            # software-pipelined emission: iter i does L1(i), L3(i-2),
            # L2(i-1); relus follow their producers.
            for i in range(NCHUNK + 3):
                # L1(i)
                if i < NCHUNK:
                    for p in range(NPAIR):
                        z1 = zpool.tile([128, T], F32, name=f"z1_{i}_{p}",
                                        tag="z")
                        for s in range(2):
                            nc.tensor.matmul(
                                z1[:, s * 512:(s + 1) * 512], w0_sb[p],
                                feat_slice(i, s),
                                start=True, stop=True)
                        h1 = h1pool.tile([128, T], BF, name=f"h1_{i}_{p}",
                                         tag="h1")
                        relu(h1, z1, b0_sb[p][:, i:i + 1])
                        h1s[(i, p)] = h1

                # L3(i-3)
                j3 = i - 3
                if j3 >= 0:
                    for p in range(NPAIR):
                        h2 = h2s[(j3, p)]
                        for s in range(2):
                            g = j3 * 8 + p * 2 + s
                            lg = g % 64
                            if lg == 0:
                                obank["tile"] = opool.tile(
                                    [128, 512], F32, name=f"ob{g // 64}",
                                    tag="ob")
                                obank["idx"] = g // 64
                            jj, jv = lg // 16, lg % 16
                            nc.tensor.matmul(
                                obank["tile"][32 * jj:32 * jj + 32, :],
                                w2_sb[p][:, 32 * jv:32 * jv + 32],
                                h2[:, s * 512:(s + 1) * 512],
                                start=(jv == 0), stop=(jv == 15),
                                tile_position=(0, 32 * jj))
                            if g == GROUPS - 1:
                                # last bank: window 0 (rows 0-31) already
                                # flushed when it completed at lg 15
                                flush_obank(32, (g % 64) * 2 + 2 - 32)
                            elif lg == 63:
                                flush_obank(0, 128)
                            elif g >= (N_BANKS - 1) * 64 and lg == 15:
                                flush_obank(0, 32)
                        if j3 >= 1:
                            h2s.pop((j3 - 1, p), None)

                # L2(i-1)
                j2 = i - 1
                if 0 <= j2 < NCHUNK:
                    for p in range(NPAIR):
                        z2 = zpool.tile([128, T], F32, name=f"z2_{j2}_{p}",
                                        tag="z")
                        h1 = h1s.pop((j2, p))
                        for s in range(2):
                            nc.tensor.matmul(z2[:, s * 512:(s + 1) * 512],
                                             w1_sb[p],
                                             h1[:, s * 512:(s + 1) * 512],
                                             start=True, stop=True)
                        h2 = h2pool.tile([128, T], BF, name=f"h2_{j2}_{p}",
                                         tag="h2")
                        relu(h2, z2, b1_sb[p])
                        h2s[(j2, p)] = h2

    nc.compile()
    return nc


def _prep_inputs(x, mask_head_params, num_ins):
    x = np.asarray(x, dtype=np.float32)
    params = np.asarray(mask_head_params, dtype=np.float32)
    num_ins = np.asarray(num_ins)
    img_idx = np.repeat(np.arange(N_IMG), num_ins)
    assert img_idx.shape[0] == N_IMG * INS_PER_IMG

    xbf = x.reshape(N_IMG, C, L).astype(bf16)

    # coord rows, periodic with T=1024: loc_x = col % 256 and the loc_y
    # in-chunk base (col // 256) % 4; chunk offsets fold into the L1 bias.
    cols = np.arange(LC)
    coords = np.stack([cols % W, (cols // W) % 4]).astype(bf16)

    in_maps = []
    for c in range(N_CORES):
        img, half = c // 2, c % 2
        inst = [img * INS_PER_IMG + k for k in range(INS_PER_IMG)]
        xbc = np.empty((CIN, LC), dtype=bf16)
        xbc[0:C] = xbf[img][:, half * LC:(half + 1) * LC]
        xbc[C:CIN] = coords
        m = {"xb": xbc}
        w0cat = np.zeros((CIN, NPAIR * 128), np.float32)
        wbf = np.zeros((128, NPAIR * (128 + 512)), np.float32)
        wf32 = np.zeros((128, NPAIR * (NCHUNK + 1) + 1), np.float32)
        for p in range(NPAIR):
            a, b = inst[2 * p], inst[2 * p + 1]
            w0_a = params[a, :L1].reshape(C, CIN)
            w0_b = params[b, :L1].reshape(C, CIN)
            # lhsT rows: [w0[:,2:].T ; w0[:,0] (loc_x) ; w0[:,1] (loc_y)]
            for k, wv in enumerate((w0_a, w0_b)):
                colsl = slice(p * 128 + 64 * k, p * 128 + 64 * k + 64)
                w0cat[0:C, colsl] = wv[:, 2:].T
                w0cat[C, colsl] = wv[:, 0]
                w0cat[C + 1, colsl] = wv[:, 1]

            # per-chunk L1 bias: b0 + w0y * (80*half + 4*i)
            b0pair = np.concatenate([params[a, B1OFF:B1OFF + C],
                                     params[b, B1OFF:B1OFF + C]])
            w0y = np.concatenate([w0_a[:, 1], w0_b[:, 1]])
            ii = np.arange(NCHUNK, dtype=np.float32)
            wf32[:, p * NCHUNK:(p + 1) * NCHUNK] = (
                b0pair[:, None] +
                w0y[:, None] * (80.0 * half + 4.0 * ii[None, :]))

            w1_a = params[a, L1:L1 + L2_].reshape(C, C)
            w1_b = params[b, L1:L1 + L2_].reshape(C, C)
            wbf[:64, p * 128:p * 128 + 64] = w1_a.T
            wbf[64:, p * 128 + 64:(p + 1) * 128] = w1_b.T

            w2_a = params[a, L1 + L2_:L1 + L2_ + C]
            w2_b = params[b, L1 + L2_:L1 + L2_ + C]
            w2pair = np.zeros((128, 2), np.float32)
            w2pair[:64, 0] = w2_a
            w2pair[64:, 1] = w2_b
            base = NPAIR * 128 + p * 512
            for j in range(16):
                wbf[:, base + 34 * j:base + 34 * j + 2] = w2pair

            wf32[:, NPAIR * NCHUNK + p] = np.concatenate(
                [params[a, B1OFF + C:B1OFF + 2 * C],
                 params[b, B1OFF + C:B1OFF + 2 * C]])

        # b2 per out-bank partition q = 32a + 16bb + (4p + 2s + m):
        # instance = 2p + m with p = (q%16)//4, m = q%2
        for q in range(128):
            p = (q % 16) // 4
            mm = q % 2
            iid = inst[2 * p + mm]
            wf32[q, NPAIR * (NCHUNK + 1)] = (params[iid, B1OFF + 2 * C]
                                             - MASK_BIAS_SHIFT)
        m["w0c"] = w0cat.astype(bf16)
        m["wbf"] = wbf.astype(bf16)
        m["wf32"] = wf32
        in_maps.append(m)
    return in_maps


def _make_runner(nc):
    """Cached jit(shard_map) executor. run_bass_via_pjrt builds a fresh
    closure per call, so jax re-lowers every time (~0.5s/call); building
    it once keeps steady-state calls at transfer+execute cost."""
    import jax
    from jax.sharding import Mesh, PartitionSpec
    from concourse import bass2jax

    bass2jax.install_neuronx_cc_hook()
    try:
        from jax.experimental.shard_map import shard_map
    except ImportError:
        shard_map = jax.shard_map

    partition_name = (nc.partition_id_tensor.name
                      if nc.partition_id_tensor else None)
    in_names, out_names, out_avals, zero_outs = [], [], [], []
    for alloc in nc.m.functions[0].allocations:
        if not isinstance(alloc, mybir.MemoryLocationSet):
            continue
        name = alloc.memorylocations[0].name
        if alloc.kind == "ExternalInput":
            if name != partition_name:
                in_names.append(name)
        elif alloc.kind == "ExternalOutput":
            shape = tuple(alloc.tensor_shape)
            dtype = mybir.dt.np(alloc.dtype)
            out_avals.append(jax.core.ShapedArray(shape, dtype))
            out_names.append(name)
            zero_outs.append(
                np.zeros((N_CORES * shape[0], *shape[1:]), dtype))
    n_params = len(in_names)
    n_outs = len(out_avals)
    all_in_names = list(in_names) + list(out_names)
    if partition_name is not None:
        all_in_names.append(partition_name)
    donate = tuple(range(n_params, n_params + n_outs))

    def _body(*args):
        operands = list(args)
        if partition_name is not None:
            operands.append(bass2jax.partition_id_tensor())
        return tuple(bass2jax._bass_exec_p.bind(
            *operands,
            out_avals=tuple(out_avals),
            in_names=tuple(all_in_names),
            out_names=tuple(out_names),
            lowering_input_output_aliases=(),
            sim_require_finite=True,
            sim_require_nnan=True,
            nc=nc,
        ))

    devices = jax.devices()[:N_CORES]
    mesh = Mesh(np.asarray(devices), ("core",))
    in_specs = (PartitionSpec("core"),) * (n_params + n_outs)
    out_specs = (PartitionSpec("core"),) * n_outs
    sharded = jax.jit(
        shard_map(_body, mesh=mesh, in_specs=in_specs,
                  out_specs=out_specs, check_rep=False),
        donate_argnums=donate, keep_unused=True)
    oi = out_names.index("o")

    def run(in_maps):
        concat_in = [np.concatenate([m[name] for m in in_maps], axis=0)
                     for name in in_names]
        out_arrs = sharded(*concat_in, *zero_outs)
        return np.asarray(out_arrs[oi]).reshape(
            N_CORES, *out_avals[oi].shape)

    return run


def kernel(x, mask_head_params, num_ins):
    if "nc" not in _cache:
        _cache["nc"] = _build_program()
        _cache["runner"] = _make_runner(_cache["nc"])
    in_maps = _prep_inputs(x, mask_head_params, num_ins)
    o_all = _cache["runner"](in_maps)
    # un-permute packed output: row b*128 + q holds (chunk 8b+2a+bb,
    # pair p, half s, inst-in-pair m) with q = 32a + 16bb + 4p + 2s + m
    q = np.arange(128)
    a, bb, cc = q // 32, (q % 32) // 16, q % 16
    p, s, m = cc // 4, (cc % 4) // 2, cc % 2
    inst_of_q = 2 * p + m
    out = np.empty((N_IMG * INS_PER_IMG, L), dtype=np.float32)
    for c in range(N_CORES):
        img, half = c // 2, c % 2
        pk = o_all[c].reshape(N_BANKS, 128, 512)
        oc = np.empty((INS_PER_IMG, LC), dtype=np.float32)
        for b in range(N_BANKS):
            chunk = 8 * b + 2 * a + bb
            valid = chunk < NCHUNK
            base = chunk * T + s * 512
            for qi in range(128):
                if valid[qi]:
                    oc[inst_of_q[qi], base[qi]:base[qi] + 512] = pk[b, qi]
        out[img * INS_PER_IMG:(img + 1) * INS_PER_IMG,
            half * LC:(half + 1) * LC] = oc
    return out.reshape(1, N_IMG * INS_PER_IMG, H, W).astype(np.float32)


# revision 55
# speedup vs baseline: 1.0048x; 1.0048x over previous
"""CondLaneHead DynamicMaskHead kernel for 8 Trainium2 NeuronCores.

Problem: per-instance 3-layer 1x1-conv MLP over a [64,160,256] feature map.
  feats = concat([loc_x, loc_y], x[img])            # [66, L], L = 160*256
  h1 = relu(w0 @ feats + b0)                        # [64, L]
  h2 = relu(w1 @ h1 + b1)                           # [64, L]
  out = w2 @ h2 + b2 - 2.19                         # [1, L]
32 instances (8 per image, 4 images).

Sharding (hybrid): core c -> image c//2, position half c%2. Each core runs
all 8 instances of its image over Lc = L/2 = 20480 positions, so every byte
of x is shipped to exactly one core.

Device mapping (per core, 8 instances = 4 pairs, 20 chunks of T=1024):
  - feats live in 10 resident SBUF tiles [66, 2048] bf16: rows 0-63 = x
    chunk (64-row DMA -> spreads across all 16 SDMA engines; odd partition
    counts serialize onto one engine at ~26 GB/s), row 64 = loc_x pattern,
    row 65 = (col//256) pattern (both periodic with 1024, loaded once).
    The chunk-dependent part of the loc_y term, w0y*(80*half + 4*i), is
    folded into a per-chunk L1 relu bias together with b0.
  - L1: per pair one [66,128] bf16 lhsT (2 instances side by side), two
    512-col matmuls per chunk into a [128,1024] PSUM tile.
  - L2: block-diagonal [128,128] bf16 weights per pair.
  - L3 (64->1): outputs packed across PSUM partitions: per 512-position
    group one matmul with a zero-padded [128,32] w2 slice writes a [32,512]
    window (tile_position), accumulating 16 groups per window, so one
    [128,512] PSUM bank collects 64 groups before a single bias+copy op
    and a few strided DMAs to DRAM.
  - relu work (PSUM->SBUF copies) is split greedily between ACT and DVE.
"""

import sys

if "/opt/trn_rl_repo" not in sys.path:
    sys.path.insert(0, "/opt/trn_rl_repo")

import numpy as np
import ml_dtypes

import concourse.bass as bass
import concourse.mybir as mybir
from concourse import bacc
from concourse.tile import TileContext
from concourse.bass_utils import run_bass_kernel_spmd

BF = mybir.dt.bfloat16
F32 = mybir.dt.float32
AT = mybir.ActivationFunctionType
AL = mybir.AluOpType
bf16 = ml_dtypes.bfloat16

# Problem geometry (hardcoded per spec)
N_IMG, INS_PER_IMG, C, H, W = 4, 8, 64, 160, 256
CIN = C + 2
L = H * W                       # 40960 positions per image
L1, L2_, L3_ = (C + 2) * C, C * C, C
B1OFF = L1 + L2_ + L3_          # offsets into the 8513-param vector
MASK_BIAS_SHIFT = 2.19

N_CORES = 8
NPAIR = 4                       # 8 instances per core, 2 per matmul pack
LC = L // 2                     # 20480 positions per core
T = 1024                        # positions per chunk
NCHUNK = LC // T                # 20
FT = 5120                       # positions per feats SBUF tile
NFT = LC // FT                  # 4 resident feats tiles
CPF = FT // T                   # 5 chunks per feats tile
GROUPS = NCHUNK * NPAIR * 2     # 160 [2, 512] position-groups per core
N_BANKS = (GROUPS + 63) // 64   # 3 output PSUM bank fills (64, 64, 32)

# relu op cost estimates (ns, HW-measured) for greedy ACT/DVE balancing
COST_DVE = 1258.0
COST_ACT = 1165.0

_cache = {}


def _build_program():
    nc = bacc.Bacc("TRN2", target_bir_lowering=False, debug=False)

    # batched inputs: DMA-issue costs ~0.6us each on the SP queue, so ship
    # few big tensors. xb rows 64/65 carry the coord patterns. wbf packs
    # w1 (4x128 cols) then w2pad (4x512 cols); wf32 packs b0 (4xNCHUNK),
    # b1 (4x1), b2 (1).
    xb = nc.dram_tensor("xb", [CIN, LC], BF, kind="ExternalInput")
    w0c = nc.dram_tensor("w0c", [CIN, NPAIR * 128], BF, kind="ExternalInput")
    wbf = nc.dram_tensor("wbf", [128, NPAIR * (128 + 512)], BF,
                         kind="ExternalInput")
    wf32 = nc.dram_tensor("wf32", [128, NPAIR * (NCHUNK + 1) + 1], F32,
                          kind="ExternalInput")
    # packed output: [bank, q, col]; host un-permutes (q encodes
    # chunk/pair/half/instance) — keeps each flush one big contiguous DMA.
    o = nc.dram_tensor("o", [N_BANKS * 128, 512], F32, kind="ExternalOutput")

    eng_ns = {"dve": 0.0, "act": 0.0}

    def relu(dst, src, bias_ap):
        if eng_ns["dve"] + COST_DVE <= eng_ns["act"] + COST_ACT:
            eng_ns["dve"] += COST_DVE
            if bias_ap is None:
                nc.vector.tensor_scalar(out=dst, in0=src, scalar1=0.0,
                                        scalar2=None, op0=AL.max)
            else:
                nc.vector.tensor_scalar(out=dst, in0=src, scalar1=bias_ap,
                                        scalar2=0.0, op0=AL.add, op1=AL.max)
        else:
            eng_ns["act"] += COST_ACT
            if bias_ap is None:
                nc.scalar.activation(dst, src, AT.Relu)
            else:
                nc.scalar.activation(dst, src, AT.Relu, bias=bias_ap)

    with TileContext(nc) as tc:
        with tc.tile_pool(name="consts", bufs=1) as cpool, \
             tc.tile_pool(name="zpool", bufs=3, space="PSUM") as zpool, \
             tc.tile_pool(name="opool", bufs=2, space="PSUM") as opool, \
             tc.tile_pool(name="h1pool", bufs=10) as h1pool, \
             tc.tile_pool(name="h2pool", bufs=16) as h2pool, \
             tc.tile_pool(name="spool", bufs=2) as spool:

            # PE warm-up: dummy matmuls on a zeroed tile fill the idle
            # window while the first DMAs land, so the p-state ramp
            # (0.65/1.2 GHz until ~3us of continuous execution) happens on
            # throwaway work instead of the first real chunks.
            warm = cpool.tile([128, 512], BF, name="warm")
            nc.vector.memset(warm, 0.0)
            wz = zpool.tile([128, T], F32, name="warmz", tag="z")
            for k in range(16):
                nc.tensor.matmul(wz[:, 0:512], warm[:, 0:128], warm,
                                 start=True, stop=True)

            # DMA order: what the first chunk needs first (w0c + a small
            # first feats piece), then relu bias (wf32), then L2/L3 weights,
            # then the rest of the feats. All on the sync ring — issuing
            # from the scalar ring delays ACT's relu work (measured slower).
            w0c_sb = cpool.tile([CIN, NPAIR * 128], BF, name="w0csb")
            nc.sync.dma_start(out=w0c_sb, in_=w0c.ap())
            ft0a = cpool.tile([CIN, T], BF, name="ft0a")
            nc.sync.dma_start(out=ft0a, in_=xb.ap()[:, 0:T])
            wf32_sb = cpool.tile([128, NPAIR * (NCHUNK + 1) + 1], F32,
                                 name="wf32sb")
            nc.sync.dma_start(out=wf32_sb, in_=wf32.ap())
            ft0b = cpool.tile([CIN, FT - T], BF, name="ft0b")
            nc.sync.dma_start(out=ft0b, in_=xb.ap()[:, T:FT])
            wbf_sb = cpool.tile([128, NPAIR * (128 + 512)], BF, name="wbfsb")
            nc.sync.dma_start(out=wbf_sb, in_=wbf.ap())
            w0_sb = [w0c_sb[:, p * 128:(p + 1) * 128] for p in range(NPAIR)]
            w1_sb = [wbf_sb[:, p * 128:(p + 1) * 128] for p in range(NPAIR)]
            w2_sb = [wbf_sb[:, NPAIR * 128 + p * 512:
                            NPAIR * 128 + (p + 1) * 512]
                     for p in range(NPAIR)]
            b0_sb = [wf32_sb[:, p * NCHUNK:(p + 1) * NCHUNK]
                     for p in range(NPAIR)]
            b1_sb = [wf32_sb[:, NPAIR * NCHUNK + p:NPAIR * NCHUNK + p + 1]
                     for p in range(NPAIR)]
            b2_sb = wf32_sb[:, NPAIR * (NCHUNK + 1):
                            NPAIR * (NCHUNK + 1) + 1]

            # remaining resident feats tiles (x + coord rows in one DMA each)
            fts = [None]
            for j in range(1, NFT):
                ft = cpool.tile([CIN, FT], BF, name=f"ft{j}")
                nc.sync.dma_start(out=ft,
                                  in_=xb.ap()[:, j * FT:(j + 1) * FT])
                fts.append(ft)

            def feat_slice(i, s):
                c = i * T + s * 512
                if c < T:
                    return ft0a[:, c:c + 512]
                if c < FT:
                    return ft0b[:, c - T:c - T + 512]
                return fts[i // CPF][:, c - (i // CPF) * FT:
                                     c - (i // CPF) * FT + 512]

            z1s, z2s, h1s, h2s = {}, {}, {}, {}
            obank = {"tile": None, "idx": -1}

            def flush_obank(row0, nrows):
                ob = obank["tile"]
                b = obank["idx"]
                stage = spool.tile([128, 512], F32, name=f"stage{b}_{row0}",
                                   tag="stage")
                nc.scalar.activation(stage[row0:row0 + nrows],
                                     ob[row0:row0 + nrows], AT.Identity,
                                     bias=b2_sb[row0:row0 + nrows])
                eng_ns["act"] += 750.0
                dst = bass.AP(o, (b * 128 + row0) * 512,
                              [[512, nrows], [1, 512]])
                nc.sync.dma_start(out=dst, in_=stage[row0:row0 + nrows])

            # software-pipelined emission: iter i does L1(i), L3(i-2),
            # L2(i-1); relus follow their producers.
            for i in range(NCHUNK + 3):
                # L1(i)
                if i < NCHUNK:
                    for p in range(NPAIR):
                        z1 = zpool.tile([128, T], F32, name=f"z1_{i}_{p}",
                                        tag="z")
                        for s in range(2):
                            nc.tensor.matmul(
                                z1[:, s * 512:(s + 1) * 512], w0_sb[p],
                                feat_slice(i, s),
                                start=True, stop=True)
                        h1 = h1pool.tile([128, T], BF, name=f"h1_{i}_{p}",
                                         tag="h1")
                        relu(h1, z1, b0_sb[p][:, i:i + 1])
                        h1s[(i, p)] = h1

                # L3(i-3)
                j3 = i - 3
                if j3 >= 0:
                    for p in range(NPAIR):
                        h2 = h2s[(j3, p)]
                        for s in range(2):
                            g = j3 * 8 + p * 2 + s
                            lg = g % 64
                            if lg == 0:
                                obank["tile"] = opool.tile(
                                    [128, 512], F32, name=f"ob{g // 64}",
                                    tag="ob")
                                obank["idx"] = g // 64
                            jj, jv = lg // 16, lg % 16
                            nc.tensor.matmul(
                                obank["tile"][32 * jj:32 * jj + 32, :],
                                w2_sb[p][:, 32 * jv:32 * jv + 32],
                                h2[:, s * 512:(s + 1) * 512],
                                start=(jv == 0), stop=(jv == 15),
                                tile_position=(0, 32 * jj))
                            if g == GROUPS - 1:
                                # last bank: window 0 (rows 0-31) already
                                # flushed when it completed at lg 15
                                flush_obank(32, (g % 64) * 2 + 2 - 32)
                            elif lg == 63:
                                flush_obank(0, 128)
                            elif g >= (N_BANKS - 1) * 64 and lg == 15:
                                flush_obank(0, 32)
                        if j3 >= 1:
                            h2s.pop((j3 - 1, p), None)

                # L2(i-1)
                j2 = i - 1
                if 0 <= j2 < NCHUNK:
                    for p in range(NPAIR):
                        z2 = zpool.tile([128, T], F32, name=f"z2_{j2}_{p}",
                                        tag="z")
                        h1 = h1s.pop((j2, p))
                        for s in range(2):
                            nc.tensor.matmul(z2[:, s * 512:(s + 1) * 512],
                                             w1_sb[p],
                                             h1[:, s * 512:(s + 1) * 512],
                                             start=True, stop=True)
                        h2 = h2pool.tile([128, T], BF, name=f"h2_{j2}_{p}",
                                         tag="h2")
                        relu(h2, z2, b1_sb[p])
                        h2s[(j2, p)] = h2

    nc.compile()
    return nc


def _prep_inputs(x, mask_head_params, num_ins):
    x = np.asarray(x, dtype=np.float32)
    params = np.asarray(mask_head_params, dtype=np.float32)
    num_ins = np.asarray(num_ins)
    img_idx = np.repeat(np.arange(N_IMG), num_ins)
    assert img_idx.shape[0] == N_IMG * INS_PER_IMG

    xbf = x.reshape(N_IMG, C, L).astype(bf16)

    # coord rows, periodic with T=1024: loc_x = col % 256 and the loc_y
    # in-chunk base (col // 256) % 4; chunk offsets fold into the L1 bias.
    cols = np.arange(LC)
    coords = np.stack([cols % W, (cols // W) % 4]).astype(bf16)

    in_maps = []
    for c in range(N_CORES):
        img, half = c // 2, c % 2
        inst = [img * INS_PER_IMG + k for k in range(INS_PER_IMG)]
        xbc = np.empty((CIN, LC), dtype=bf16)
        xbc[0:C] = xbf[img][:, half * LC:(half + 1) * LC]
        xbc[C:CIN] = coords
        m = {"xb": xbc}
        w0cat = np.zeros((CIN, NPAIR * 128), np.float32)
        wbf = np.zeros((128, NPAIR * (128 + 512)), np.float32)
        wf32 = np.zeros((128, NPAIR * (NCHUNK + 1) + 1), np.float32)
        for p in range(NPAIR):
            a, b = inst[2 * p], inst[2 * p + 1]
            w0_a = params[a, :L1].reshape(C, CIN)
            w0_b = params[b, :L1].reshape(C, CIN)
            # lhsT rows: [w0[:,2:].T ; w0[:,0] (loc_x) ; w0[:,1] (loc_y)]
            for k, wv in enumerate((w0_a, w0_b)):
                colsl = slice(p * 128 + 64 * k, p * 128 + 64 * k + 64)
                w0cat[0:C, colsl] = wv[:, 2:].T
                w0cat[C, colsl] = wv[:, 0]
                w0cat[C + 1, colsl] = wv[:, 1]

            # per-chunk L1 bias: b0 + w0y * (80*half + 4*i)
            b0pair = np.concatenate([params[a, B1OFF:B1OFF + C],
                                     params[b, B1OFF:B1OFF + C]])
            w0y = np.concatenate([w0_a[:, 1], w0_b[:, 1]])
            ii = np.arange(NCHUNK, dtype=np.float32)
            wf32[:, p * NCHUNK:(p + 1) * NCHUNK] = (
                b0pair[:, None] +
                w0y[:, None] * (80.0 * half + 4.0 * ii[None, :]))

            w1_a = params[a, L1:L1 + L2_].reshape(C, C)
            w1_b = params[b, L1:L1 + L2_].reshape(C, C)
            wbf[:64, p * 128:p * 128 + 64] = w1_a.T
            wbf[64:, p * 128 + 64:(p + 1) * 128] = w1_b.T

            w2_a = params[a, L1 + L2_:L1 + L2_ + C]
            w2_b = params[b, L1 + L2_:L1 + L2_ + C]
            w2pair = np.zeros((128, 2), np.float32)
            w2pair[:64, 0] = w2_a
            w2pair[64:, 1] = w2_b
            base = NPAIR * 128 + p * 512
            for j in range(16):
                wbf[:, base + 34 * j:base + 34 * j + 2] = w2pair

            wf32[:, NPAIR * NCHUNK + p] = np.concatenate(
                [params[a, B1OFF + C:B1OFF + 2 * C],
                 params[b, B1OFF + C:B1OFF + 2 * C]])

        # b2 per out-bank partition q = 32a + 16bb + (4p + 2s + m):
        # instance = 2p + m with p = (q%16)//4, m = q%2
        for q in range(128):
            p = (q % 16) // 4
            mm = q % 2
            iid = inst[2 * p + mm]
            wf32[q, NPAIR * (NCHUNK + 1)] = (params[iid, B1OFF + 2 * C]
                                             - MASK_BIAS_SHIFT)
        m["w0c"] = w0cat.astype(bf16)
        m["wbf"] = wbf.astype(bf16)
        m["wf32"] = wf32
        in_maps.append(m)
    return in_maps


def _make_runner(nc):
    """Cached jit(shard_map) executor. run_bass_via_pjrt builds a fresh
    closure per call, so jax re-lowers every time (~0.5s/call); building
    it once keeps steady-state calls at transfer+execute cost."""
    import jax
    from jax.sharding import Mesh, PartitionSpec
    from concourse import bass2jax

    bass2jax.install_neuronx_cc_hook()
    try:
        from jax.experimental.shard_map import shard_map
    except ImportError:
        shard_map = jax.shard_map

    partition_name = (nc.partition_id_tensor.name
                      if nc.partition_id_tensor else None)
    in_names, out_names, out_avals, zero_outs = [], [], [], []
    for alloc in nc.m.functions[0].allocations:
        if not isinstance(alloc, mybir.MemoryLocationSet):
            continue
        name = alloc.memorylocations[0].name
        if alloc.kind == "ExternalInput":
            if name != partition_name:
                in_names.append(name)
        elif alloc.kind == "ExternalOutput":
            shape = tuple(alloc.tensor_shape)
            dtype = mybir.dt.np(alloc.dtype)
            out_avals.append(jax.core.ShapedArray(shape, dtype))
            out_names.append(name)
            zero_outs.append(
                np.zeros((N_CORES * shape[0], *shape[1:]), dtype))
    n_params = len(in_names)
    n_outs = len(out_avals)
    all_in_names = list(in_names) + list(out_names)
    if partition_name is not None:
        all_in_names.append(partition_name)
    donate = tuple(range(n_params, n_params + n_outs))

    def _body(*args):
        operands = list(args)
        if partition_name is not None:
            operands.append(bass2jax.partition_id_tensor())
        return tuple(bass2jax._bass_exec_p.bind(
            *operands,
            out_avals=tuple(out_avals),
            in_names=tuple(all_in_names),
            out_names=tuple(out_names),
            lowering_input_output_aliases=(),
            sim_require_finite=True,
            sim_require_nnan=True,
            nc=nc,
        ))

    devices = jax.devices()[:N_CORES]
    mesh = Mesh(np.asarray(devices), ("core",))
    in_specs = (PartitionSpec("core"),) * (n_params + n_outs)
    out_specs = (PartitionSpec("core"),) * n_outs
    sharded = jax.jit(
        shard_map(_body, mesh=mesh, in_specs=in_specs,
                  out_specs=out_specs, check_rep=False),
        donate_argnums=donate, keep_unused=True)
    oi = out_names.index("o")

    def run(in_maps):
        concat_in = [np.concatenate([m[name] for m in in_maps], axis=0)
                     for name in in_names]
        out_arrs = sharded(*concat_in, *zero_outs)
        return np.asarray(out_arrs[oi]).reshape(
            N_CORES, *out_avals[oi].shape)

    return run


def kernel(x, mask_head_params, num_ins):
    if "nc" not in _cache:
        _cache["nc"] = _build_program()
        _cache["runner"] = _make_runner(_cache["nc"])
    in_maps = _prep_inputs(x, mask_head_params, num_ins)
    o_all = _cache["runner"](in_maps)
    # un-permute packed output: row b*128 + q holds (chunk 8b+2a+bb,
    # pair p, half s, inst-in-pair m) with q = 32a + 16bb + 4p + 2s + m
    q = np.arange(128)
    a, bb, cc = q // 32, (q % 32) // 16, q % 16
    p, s, m = cc // 4, (cc % 4) // 2, cc % 2
    inst_of_q = 2 * p + m
    out = np.empty((N_IMG * INS_PER_IMG, L), dtype=np.float32)
    for c in range(N_CORES):
        img, half = c // 2, c % 2
        pk = o_all[c].reshape(N_BANKS, 128, 512)
        oc = np.empty((INS_PER_IMG, LC), dtype=np.float32)
        for b in range(N_BANKS):
            chunk = 8 * b + 2 * a + bb
            valid = chunk < NCHUNK
            base = chunk * T + s * 512
            for qi in range(128):
                if valid[qi]:
                    oc[inst_of_q[qi], base[qi]:base[qi] + 512] = pk[b, qi]
        out[img * INS_PER_IMG:(img + 1) * INS_PER_IMG,
            half * LC:(half + 1) * LC] = oc
    return out.reshape(1, N_IMG * INS_PER_IMG, H, W).astype(np.float32)
